# revision 7
# baseline (speedup 1.0000x reference)
"""DeepFM-style (fm1 + fm2 + DNN) Trainium2 kernel, batch-sharded across 8 NeuronCores.

Strategy
--------
Each core handles 2048 batch rows with a full replica of the (merged fm1+fm2)
embedding table, so no collectives are needed.  The random-access embedding
lookup is the bottleneck (memory regime): SWDGE descriptor generation costs
~8ns/row (~2ns across the 4 parallel SWDGE queues) plus ~1us of
engine-serial overhead per dma_gather instruction, so the design minimizes
both instruction count and scanned index positions.  dma_gather needs int16
indices, so the gather runs in two phases per "round" (half the batch x half
the feature set, giving 4 rounds per core):

  phase 1  HBM -> SBUF : rows bucketed into static 32767-row windows of the
           flat [26*100000, 128]-bf16 table (~40 windows per feature-half);
           one compact dma_gather per window (fixed CAP positions, padded
           with index 0; pads land in staging slots nothing references),
           landing window-sorted in a [128, 164, 128] staging tile
           (position i -> partition i%128, col i//128).
  phase 2  SBUF -> SBUF: ONE transposed SBUF-source dma_gather per round
           un-sorts by rank (host-computed position of each (b,f) entry) and
           lands feature-major [128 dims, 13*1024 cols] directly; staging
           ranks stay < 164*128 = 20992, inside int16.

Table rows place each f's 16 embedding dims + fm1 scalar at a per-chunk
partition slot, so 6-7 features pack into one 128-partition K-chunk with
plain DVE adds; the whole model is then PSUM-accumulated matmuls
(bf16 inputs, f32 accumulation; the fm2 cancellation runs in f32).
"""

import numpy as np
import ml_dtypes

from contextlib import ExitStack

import concourse.tile as tile
from concourse import bass, mybir, bacc
from concourse.library_config import mlp
from concourse.bass_utils import run_bass_kernel_spmd

# model dims
B, D_DENSE, F, V, ED, REP = 16384, 13, 26, 100000, 16, 64
H1, H2 = 256, 128
NCORES = 8
BL = B // NCORES            # 2048 local batch rows per core
HB = BL // 2                # 1024 rows per half
NQ = 4                      # compute quarters per core (matmul N=512)
QB = BL // NQ               # 512
FV = F * V                  # 2.6M table rows
ROWW = 128                  # bf16 elements per table row (256 B)
WROWS = 32767               # int16-addressable window size
CAP = 432                   # idx budget per (round, window); %16, +5.4 sigma
CPW = -(-CAP // 128)        # staging columns per window (4)
# feature-halves and their window ranges
FH = 13                     # features per feature-half
G_WLO = [0, (FH * V) // WROWS]                        # [0, 39]
G_WHI = [(FH * V - 1) // WROWS, (FV - 1) // WROWS]    # [39, 79]
G_NWIN = [G_WHI[0] - G_WLO[0] + 1, G_WHI[1] - G_WLO[1] + 1]   # [40, 41]
SCOLS = max(G_NWIN) * CPW   # 164 staging columns
NWI = 2 * (G_NWIN[0] + G_NWIN[1])   # total phase-1 window instructions (162)
P2N = FH * HB               # positions per phase-2 instruction (13312, %128)
# K-chunks: features per chunk and chunk base feature
CH_BASE = [0, 7, 13, 20]
CH_NF = [7, 6, 7, 6]
SLOTW = 18                  # partition slot width: 16 dims + fm1 + pad
NCHUNK = 4
KDENSE = D_DENSE + 1 + REP  # 78: dense_x rows + ones row + rep rows

BF16 = mybir.dt.bfloat16
F32 = mybir.dt.float32
I16 = mybir.dt.int16
I32 = mybir.dt.int32


def _slot_of(f):
    for c in range(NCHUNK):
        if CH_BASE[c] <= f < CH_BASE[c] + CH_NF[c]:
            return f - CH_BASE[c]
    raise ValueError(f)


def _split_waits(nc, max_waits=1):
    """walrus here encodes at most one sync wait per instruction; hoist extras
    onto InstEventSemaphore carriers on the same engine."""
    for f in nc.m.functions:
        for bb in f.blocks:
            new_insts = []
            for inst in bb.instructions:
                si = inst.sync_info
                if si and si.on_wait and len(si.on_wait) > max_waits:
                    waits = list(si.on_wait)
                    for i, w in enumerate(waits[:-max_waits]):
                        ev = mybir.InstEventSemaphore(
                            name=f"{inst.name}-waitsplit{i}", ins=[], outs=[])
                        ev.engine = inst.engine
                        ev.sync_info = mybir.SyncInfo(on_wait=[w], on_update=[])
                        new_insts.append(ev)
                    si.on_wait = waits[-max_waits:]
                new_insts.append(inst)
            bb.instructions[:] = new_insts


def _build_program():
    nc = bacc.Bacc("TRN2", num_swdge_queues=4)

    table = nc.declare_dram_parameter("table", [FV, ROWW], BF16, isOutput=False)
    ph1_idx = nc.declare_dram_parameter(
        "ph1_idx", [128, NWI * (CAP // 16)], I16, isOutput=False)
    ph2_idx = nc.declare_dram_parameter(
        "ph2_idx", [128, 4 * (P2N // 16)], I16, isOutput=False)
    densrep = nc.declare_dram_parameter("densrep", [KDENSE, BL], F32, isOutput=False)
    w1c = nc.declare_dram_parameter("w1c", [NCHUNK * 128, H1], BF16, isOutput=False)
    w1d = nc.declare_dram_parameter("w1d", [KDENSE, H1], F32, isOutput=False)
    fm1w = nc.declare_dram_parameter("fm1w", [KDENSE, 1], F32, isOutput=False)
    sel = nc.declare_dram_parameter("sel", [128, 17], BF16, isOutput=False)
    negmask = nc.declare_dram_parameter("negmask", [128, 1], BF16, isOutput=False)
    halves16 = nc.declare_dram_parameter("halves16", [16, 1], F32, isOutput=False)
    e17 = nc.declare_dram_parameter("e17", [17, 1], F32, isOutput=False)
    w2 = nc.declare_dram_parameter("w2", [H1, H2], BF16, isOutput=False)
    b2row = nc.declare_dram_parameter("b2row", [1, H2], BF16, isOutput=False)
    fw = nc.declare_dram_parameter("fw", [H2, 1], BF16, isOutput=False)
    out = nc.declare_dram_parameter("out", [1, BL], F32, isOutput=True)

    with tile.TileContext(nc) as tc, ExitStack() as ctx:
        cpool = ctx.enter_context(tc.tile_pool(name="const", bufs=1))
        stgpool = ctx.enter_context(tc.tile_pool(name="stg", bufs=1))
        xgpool = ctx.enter_context(tc.tile_pool(name="xg", bufs=1))
        xcpool = ctx.enter_context(tc.tile_pool(name="xc", bufs=1))
        hpool = ctx.enter_context(tc.tile_pool(name="h", bufs=2))
        spool = ctx.enter_context(tc.tile_pool(name="scratch", bufs=2))
        ppool = ctx.enter_context(tc.tile_pool(name="psum", bufs=1, space="PSUM"))
        ppool2 = ctx.enter_context(tc.tile_pool(name="psum2", bufs=2, space="PSUM"))

        nc.gpsimd.load_library(mlp)

        # constants / weights into SBUF
        ph1_idx_t = cpool.tile([128, NWI * (CAP // 16)], I16)
        nc.sync.dma_start(out=ph1_idx_t[:], in_=ph1_idx[:])
        ph2_idx_t = cpool.tile([128, 4 * (P2N // 16)], I16)
        nc.sync.dma_start(out=ph2_idx_t[:], in_=ph2_idx[:])
        densrep_t = cpool.tile([KDENSE, BL], F32)
        nc.sync.dma_start(out=densrep_t[:], in_=densrep[:])
        w1c_t = []
        for g in range(NCHUNK):
            t = cpool.tile([128, H1], BF16, tag=f"w1c{g}")
            nc.sync.dma_start(out=t[:], in_=w1c[g * 128:(g + 1) * 128, :])
            w1c_t.append(t)
        w1d_t = cpool.tile([KDENSE, H1], F32)
        nc.sync.dma_start(out=w1d_t[:], in_=w1d[:])
        fm1w_t = cpool.tile([KDENSE, 1], F32)
        nc.sync.dma_start(out=fm1w_t[:], in_=fm1w[:])
        sel_t = cpool.tile([128, 17], BF16)
        nc.sync.dma_start(out=sel_t[:], in_=sel[:])
        negmask_t = cpool.tile([128, 1], BF16)
        nc.sync.dma_start(out=negmask_t[:], in_=negmask[:])
        halves16_t = cpool.tile([16, 1], F32)
        nc.sync.dma_start(out=halves16_t[:], in_=halves16[:])
        e17_t = cpool.tile([17, 1], F32)
        nc.sync.dma_start(out=e17_t[:], in_=e17[:])
        w2_t = []
        for g in range(2):
            t = cpool.tile([128, H2], BF16, tag=f"w2{g}")
            nc.sync.dma_start(out=t[:], in_=w2[g * 128:(g + 1) * 128, :])
            w2_t.append(t)
        b2row_t = cpool.tile([1, H2], BF16)
        nc.sync.dma_start(out=b2row_t[:], in_=b2row[:])
        fw_t = cpool.tile([H2, 1], BF16)
        nc.sync.dma_start(out=fw_t[:], in_=fw[:])
        onesbf = cpool.tile([1, QB], BF16)
        nc.vector.memset(onesbf[:], 1.0)

        wi = 0        # running phase-1 window-instruction index
        qno = 0       # running SWDGE queue rotation
        for h in range(2):
            xc = [None] * NCHUNK
            x2 = [None] * NCHUNK
            for G in range(2):
                nwin = G_NWIN[G]
                wlo = G_WLO[G]
                # ---- phase 1: windowed compact gather HBM -> staging ----
                stg = stgpool.tile([128, SCOLS, ROWW], BF16, tag="stg")
                for w in range(nwin):
                    base = (wlo + w) * WROWS
                    nrows = min(WROWS, FV - base)
                    nc.gpsimd.dma_gather(
                        out_ap=stg[:, w * CPW:(w + 1) * CPW, :],
                        in_ap=table[base:base + nrows, :],
                        idxs_ap=ph1_idx_t[:, wi * (CAP // 16):(wi + 1) * (CAP // 16)],
                        num_idxs=CAP,
                        num_idxs_reg=CAP,
                        elem_size=ROWW,
                        single_packet=True,
                        queue_num=qno % 4,
                    )
                    wi += 1
                    qno += 1
                # ---- phase 2: one SBUF-source transposed un-sort gather ----
                r = 2 * h + G
                xg = xgpool.tile([128, 1, P2N], BF16, tag="xg")
                nc.gpsimd.dma_gather(
                    out_ap=xg[:],
                    in_ap=stg[:],
                    idxs_ap=ph2_idx_t[:, r * (P2N // 16):(r + 1) * (P2N // 16)],
                    num_idxs=P2N,
                    num_idxs_reg=P2N,
                    elem_size=ROWW,
                    single_packet=False,
                    transpose=True,
                    sbuf_tokens_per_rank=128,
                    sbuf_free_dim_per_rank=ROWW * 2,
                    sbuf_free_dim_pad_per_rank=0,
                    sbuf_byte_offset=0,
                    queue_num=qno % 4,
                )
                qno += 1
                # ---- pack this feature-half's two K-chunks ----
                for c in (2 * G, 2 * G + 1):
                    nf = CH_NF[c]
                    fis = [CH_BASE[c] + s - G * FH for s in range(nf)]  # fi in xg
                    t = xcpool.tile([128, HB], BF16, tag=f"xc{c}")
                    nc.vector.tensor_add(
                        out=t[:],
                        in0=xg[:, 0, fis[0] * HB:(fis[0] + 1) * HB],
                        in1=xg[:, 0, fis[1] * HB:(fis[1] + 1) * HB])
                    for fi in fis[2:]:
                        nc.vector.tensor_add(
                            out=t[:], in0=t[:],
                            in1=xg[:, 0, fi * HB:(fi + 1) * HB])
                    xc[c] = t
                    t2 = xcpool.tile([128, HB], BF16, tag=f"x2{c}")
                    nc.vector.tensor_mul(out=t2[:], in0=t[:], in1=t[:])
                    x2[c] = t2

            # ---- compute the two quarters of this half ----
            for qq in range(2):
                q = 2 * h + qq
                csl = slice(qq * QB, (qq + 1) * QB)
                dr_c = densrep_t[:, q * QB:(q + 1) * QB]

                h1p = []
                for hh in range(2):
                    p = ppool.tile([128, QB], F32, tag=f"h1p{hh}")
                    for g in range(NCHUNK):
                        nc.tensor.matmul(
                            out=p[:], lhsT=w1c_t[g][:, hh * 128:(hh + 1) * 128],
                            rhs=xc[g][:, csl], start=(g == 0), stop=False)
                    nc.tensor.matmul(
                        out=p[:], lhsT=w1d_t[:, hh * 128:(hh + 1) * 128], rhs=dr_c,
                        start=False, stop=True)
                    h1p.append(p)
                h1sb = []
                for hh in range(2):
                    t = hpool.tile([128, QB], BF16, tag=f"h1sb{hh}")
                    nc.scalar.activation(
                        out=t[:], in_=h1p[hh][:],
                        func=mybir.ActivationFunctionType.Relu)
                    h1sb.append(t)

                sepsum = ppool.tile([17, QB], F32, tag="sepsum")
                for g in range(NCHUNK):
                    nc.tensor.matmul(out=sepsum[:], lhsT=sel_t[:], rhs=xc[g][:, csl],
                                     start=(g == 0), stop=(g == NCHUNK - 1))
                se_sb = spool.tile([17, QB], F32, tag="se_sb")
                nc.vector.tensor_copy(out=se_sb[:], in_=sepsum[:])
                se2_sb = spool.tile([16, QB], F32, tag="se2_sb")
                nc.vector.tensor_mul(out=se2_sb[:], in0=se_sb[0:16, :],
                                     in1=se_sb[0:16, :])

                h2p = ppool.tile([128, QB], F32, tag="h2p")
                nc.tensor.matmul(out=h2p[:], lhsT=w2_t[0][:], rhs=h1sb[0][:],
                                 start=True, stop=False)
                nc.tensor.matmul(out=h2p[:], lhsT=w2_t[1][:], rhs=h1sb[1][:],
                                 start=False, stop=False)
                nc.tensor.matmul(out=h2p[:], lhsT=b2row_t[:], rhs=onesbf[:],
                                 start=False, stop=True)
                h2sb = hpool.tile([128, QB], BF16, tag="h2sb")
                nc.scalar.activation(
                    out=h2sb[:], in_=h2p[:], func=mybir.ActivationFunctionType.Relu)

                op = ppool2.tile([1, QB], F32, tag="outp")
                for g in range(NCHUNK):                     # -0.5 * sum e^2
                    nc.tensor.matmul(out=op[:], lhsT=negmask_t[:], rhs=x2[g][:, csl],
                                     start=(g == 0), stop=False)
                nc.tensor.matmul(out=op[:], lhsT=halves16_t[:], rhs=se2_sb[:],
                                 start=False, stop=False)   # +0.5*sum se^2
                nc.tensor.matmul(out=op[:], lhsT=e17_t[:], rhs=se_sb[:],
                                 start=False, stop=False)   # + fm1 sparse
                nc.tensor.matmul(out=op[:], lhsT=fm1w_t[:], rhs=dr_c,
                                 start=False, stop=False)   # + fm1 dense + biases
                nc.tensor.matmul(out=op[:], lhsT=fw_t[:], rhs=h2sb[:],
                                 start=False, stop=True)    # + dnn out
                osb = spool.tile([1, QB], F32, tag="osb")
                nc.vector.tensor_copy(out=osb[:], in_=op[:])
                nc.sync.dma_start(out=out[0:1, q * QB:(q + 1) * QB], in_=osb[:])

    nc.compile()
    _split_waits(nc)
    return nc


_PROGRAM_CACHE = {}


def _get_program():
    if "nc" not in _PROGRAM_CACHE:
        _PROGRAM_CACHE["nc"] = _build_program()
    return _PROGRAM_CACHE["nc"]


def _prep_shared(fm1_tables, fm2_tables, fm1_dense_w, fm1_dense_b,
                 dnn_w1, dnn_b1, dnn_w2, dnn_b2, final_w, final_b):
    bf16 = ml_dtypes.bfloat16
    # merged table: row f*V+id holds e-dims at slot_of(f)*SLOTW, fm1 at +16
    tab = np.zeros((FV, ROWW), dtype=bf16)
    fm2 = np.ascontiguousarray(fm2_tables, dtype=np.float32).reshape(F, V, ED)
    fm1 = np.ascontiguousarray(fm1_tables, dtype=np.float32).reshape(F, V)
    for f in range(F):
        s = _slot_of(f) * SLOTW
        tab[f * V:(f + 1) * V, s:s + ED] = fm2[f].astype(bf16)
        tab[f * V:(f + 1) * V, s + ED] = fm1[f].astype(bf16)

    # W1 chunks: chunk c row s*SLOTW+d  <->  w1 row (CH_BASE[c]+s)*ED+d
    w1 = np.asarray(dnn_w1, dtype=np.float32)
    w1c = np.zeros((NCHUNK * 128, H1), dtype=bf16)
    for c in range(NCHUNK):
        for s in range(CH_NF[c]):
            f = CH_BASE[c] + s
            w1c[c * 128 + s * SLOTW:c * 128 + s * SLOTW + ED, :] = \
                w1[f * ED:(f + 1) * ED, :].astype(bf16)

    # dense K-chunk rows: [dense_x(13) | ones(1) | rep(64)]
    w1d = np.zeros((KDENSE, H1), dtype=np.float32)
    w1d[0:D_DENSE, :] = w1[F * ED:F * ED + D_DENSE, :]
    w1d[D_DENSE, :] = np.asarray(dnn_b1, dtype=np.float32)
    w1d[D_DENSE + 1:, :] = w1[F * ED + D_DENSE:, :]

    fm1w = np.zeros((KDENSE, 1), dtype=np.float32)
    fdw = np.asarray(fm1_dense_w, dtype=np.float32).reshape(-1)
    fm1w[0:D_DENSE, 0] = fdw[0:D_DENSE]
    fm1w[D_DENSE, 0] = float(np.asarray(fm1_dense_b).reshape(-1)[0]) + \
        float(np.asarray(final_b).reshape(-1)[0])
    fm1w[D_DENSE + 1:, 0] = fdw[D_DENSE:]

    sel = np.zeros((128, 17), dtype=bf16)
    for k in range(128):
        rr = k % SLOTW
        if rr < ED:
            sel[k, rr] = 1.0
        elif rr == ED:
            sel[k, 16] = 1.0
    negmask = np.zeros((128, 1), dtype=bf16)
    for k in range(128):
        if k % SLOTW < ED:
            negmask[k, 0] = -0.5
    halves16 = np.full((16, 1), 0.5, dtype=np.float32)
    e17 = np.zeros((17, 1), dtype=np.float32)
    e17[16, 0] = 1.0

    return dict(
        table=tab, w1c=w1c, w1d=w1d, fm1w=fm1w, sel=sel, negmask=negmask,
        halves16=halves16, e17=e17,
        w2=np.asarray(dnn_w2, dtype=np.float32).astype(bf16),
        b2row=np.asarray(dnn_b2, dtype=np.float32).reshape(1, H2).astype(bf16),
        fw=np.asarray(final_w, dtype=np.float32).reshape(H2, 1).astype(bf16),
    )


def _wrap16(a):
    """[N, P] int16 position arrays -> [128, N*P/16] wrapped+replicated layout."""
    n, p = a.shape
    w = a.reshape(n, p // 16, 16).transpose(0, 2, 1)       # [n, 16, p/16]
    return np.tile(w, (1, 8, 1)).transpose(1, 0, 2).reshape(128, n * (p // 16))


def _prep_core(sparse_ids, dense_x, representation):
    """Per-core index/layout prep: phase-1 window buckets + phase-2 ranks."""
    ids = np.asarray(sparse_ids, dtype=np.int64)              # [BL, F]
    flat = ids + (np.arange(F, dtype=np.int64) * V)[None, :]
    w_e = flat // WROWS                                       # window per entry
    lidx = (flat - w_e * WROWS).astype(np.int16)
    h_e = (np.arange(BL, dtype=np.int64) // HB)[:, None].repeat(F, axis=1)
    G_e = (np.arange(F, dtype=np.int64) // FH)[None, :].repeat(BL, axis=0)

    # instruction index per (h, G, w)
    winst = np.zeros((2, 2, 82), dtype=np.int64)
    wi = 0
    for h in range(2):
        for G in range(2):
            for w in range(G_NWIN[G]):
                winst[h, G, w] = wi
                wi += 1
    assert wi == NWI

    hf = h_e.reshape(-1)
    Gf = G_e.reshape(-1)
    wf = w_e.reshape(-1)
    lf = lidx.reshape(-1)
    wrel = wf - np.array(G_WLO, dtype=np.int64)[Gf]
    key = winst[hf, Gf, wrel]

    order = np.lexsort((key,))
    ks = key[order]
    uniq, first_pos, counts = np.unique(ks, return_index=True, return_counts=True)
    j = np.arange(len(ks)) - np.repeat(first_pos, counts)
    # entries beyond CAP overflow (prob ~1e-4): alias them to rank 0
    ovf = j >= CAP
    jc = np.minimum(j, CAP - 1)

    ph1_pos = np.zeros((NWI, CAP), dtype=np.int16)   # pad = idx 0 (harmless)
    keep = ~ovf
    ph1_pos[ks[keep], jc[keep]] = lf[order][keep]

    rank = np.zeros(BL * F, dtype=np.int32)
    rnk = wrel[order] * (CPW * 128) + jc
    rnk[ovf] = 0
    rank[order] = rnk

    ph1_wrapped = _wrap16(ph1_pos)

    # phase-2 idx: round r = 2h+G, position fi*HB + bb -> rank of (b, f)
    rank2 = rank.reshape(BL, F)
    ph2_pos = np.zeros((4, P2N), dtype=np.int16)
    for h in range(2):
        for G in range(2):
            blk = rank2[h * HB:(h + 1) * HB, G * FH:(G + 1) * FH]  # [HB, FH]
            ph2_pos[2 * h + G] = blk.T.reshape(-1).astype(np.int16)
    ph2_wrapped = _wrap16(ph2_pos)

    densrep = np.empty((KDENSE, BL), dtype=np.float32)
    densrep[0:D_DENSE] = np.asarray(dense_x, dtype=np.float32).T
    densrep[D_DENSE] = 1.0
    densrep[D_DENSE + 1:] = np.asarray(representation, dtype=np.float32).T

    return dict(ph1_idx=ph1_wrapped, ph2_idx=ph2_wrapped, densrep=densrep)


def kernel(representation, dense_x, sparse_ids, fm1_tables, fm2_tables,
           fm1_dense_w, fm1_dense_b, dnn_w1, dnn_b1, dnn_w2, dnn_b2,
           final_w, final_b):
    nc = _get_program()
    shared = _prep_shared(fm1_tables, fm2_tables, fm1_dense_w, fm1_dense_b,
                          dnn_w1, dnn_b1, dnn_w2, dnn_b2, final_w, final_b)
    in_maps = []
    for i in range(NCORES):
        sl = slice(i * BL, (i + 1) * BL)
        core = _prep_core(np.asarray(sparse_ids)[sl],
                          np.asarray(dense_x)[sl],
                          np.asarray(representation)[sl])
        in_maps.append({**shared, **core})
    res = run_bass_kernel_spmd(nc, in_maps, core_ids=list(range(NCORES)))
    out = np.concatenate(
        [res.results[i]["out"].reshape(-1) for i in range(NCORES)])
    return out.reshape(B, 1).astype(np.float32)
